# revision 6
# baseline (speedup 1.0000x reference)
"""Trainium2 Bass kernel: 4096x4096 valid 5x5 cross-correlation + scalar bias.

Strategy (8 NeuronCores, SPMD):
  - Shard the OUTPUT by columns: core c computes out[:, 512c : 512c+512]
    (core 7's last 4 columns are padding, trimmed after gather). Each core
    reads x rows 0..4095, cols [512c, 512c+516) (host-padded to width 4100).
  - On-core: the 5x5 conv is computed as banded-matrix matmuls on the
    TensorEngine. For an input row-tile X_g = x[124g : 124g+128, :] and
    kernel column dj, the banded matrix B_dj[k, m] = w[k-m, dj] gives
      (B_dj^T @ X_g[:, dj:dj+512])[m, n] = sum_di w[di, dj] x[124g+m+di, n+dj]
    so accumulating the 5 dj-matmuls in PSUM yields 124 valid output rows
    per tile. 4092 = 33 * 124 exactly; 33 tiles cover rows 0..4095 exactly.
  - Matmuls run in float32r (single-pass fp32, ~1e-4 rel err) at 4x the
    fp32 matmul rate. PSUM accumulation is fp32. Bias is fused into the
    PSUM->SBUF drain (ScalarE Identity-activation / VectorE tensor_scalar).
"""
import os

os.environ.setdefault("MYCRO_LOCAL_CACHE", "1")

import numpy as np

import concourse.bass as bass
import concourse.bacc as bacc
import concourse.tile as tile
import concourse.mybir as mybir
from concourse import bass_utils

H, W = 4096, 4096
KH, KW = 5, 5
OH, OW = H - KH + 1, W - KW + 1          # 4092, 4092
NCORES = 8
COLS = 512                               # output cols per core
XC = COLS + KW - 1                       # 516 input cols per core
NG = 33                                  # row tiles per core (33*124 = 4092)
RV = 124                                 # valid output rows per tile
BLK = 8                                  # tiles per PSUM block (8 banks)

_compiled = None
TRACE = False            # test harness can flip this for neuron-profile timing
LAST_EXEC_NS = None


def _build():
    nc = bacc.Bacc("TRN2", target_bir_lowering=False, debug=False,
                   num_devices=NCORES)

    x_dram = nc.dram_tensor("xs", (H, XC), mybir.dt.float32r,
                            kind="ExternalInput")
    b_dram = nc.dram_tensor("bmat", (128, KW * 128), mybir.dt.float32r,
                            kind="ExternalInput")
    bias_dram = nc.dram_tensor("biast", (128, 1), mybir.dt.float32,
                               kind="ExternalInput")
    out_dram = nc.dram_tensor("out", (OH, COLS), mybir.dt.float32,
                              kind="ExternalOutput")

    blocks = [list(range(s, min(s + BLK, NG))) for s in range(0, NG, BLK)]

    with tile.TileContext(nc) as tc:
        with (
            tc.tile_pool(name="const", bufs=1) as cpool,
            tc.tile_pool(name="x", bufs=NG) as xpool,
            tc.tile_pool(name="stage", bufs=2) as spool,
            tc.tile_pool(name="psum", bufs=BLK, space=bass.MemorySpace.PSUM) as ppool,
        ):
            bt = cpool.tile([128, KW * 128], mybir.dt.float32r)
            biast = cpool.tile([128, 1], mybir.dt.float32)
            nc.sync.dma_start(bt[:], b_dram.ap())
            nc.sync.dma_start(biast[:], bias_dram.ap())

            # input row-tiles: tile g holds x rows [124g, 124g+128)
            xts = []
            for g in range(NG):
                xt = xpool.tile([128, XC], mybir.dt.float32r, tag="x")
                nc.sync.dma_start(xt[:], x_dram.ap()[124 * g:124 * g + 128, :])
                xts.append(xt)

            for blk in blocks:
                stg = spool.tile([128, len(blk) * COLS], mybir.dt.float32)
                psts = {}
                for g in blk:
                    psts[g] = ppool.tile([128, COLS], mybir.dt.float32,
                                         name=f"ps{g}", tag="ps")
                # weight-stationary sweep: dj outer, tiles inner
                for dj in range(KW):
                    for g in blk:
                        nc.tensor.matmul(
                            psts[g][:],
                            bt[:, dj * 128:(dj + 1) * 128],
                            xts[g][:, dj:dj + COLS],
                            start=(dj == 0),
                            stop=(dj == KW - 1),
                        )
                # drain PSUM -> stage with fused bias, split ACT/DVE
                for i, g in enumerate(blk):
                    dst = stg[0:RV, i * COLS:(i + 1) * COLS]
                    if i % 8 < 5:
                        nc.vector.tensor_scalar_add(dst, psts[g][0:RV, :],
                                                    biast[0:RV, :])
                    else:
                        nc.scalar.activation(dst, psts[g][0:RV, :],
                                             mybir.ActivationFunctionType.Identity,
                                             bias=biast[0:RV, :])
                # one DMA for the whole block: stage[p, (g, c)] ->
                # out rows [124g + p], cols c
                g0 = blk[0]
                dst_ap = out_dram.ap()[124 * g0:124 * g0 + RV * len(blk), :]
                dst_ap = dst_ap.rearrange("(g p) c -> p g c", p=RV)
                src_ap = stg[0:RV, :].rearrange("p (g c) -> p g c", g=len(blk))
                nc.scalar.dma_start(dst_ap, src_ap)

    nc.compile()
    return nc


def _banded(weight: np.ndarray) -> np.ndarray:
    ball = np.zeros((128, KW * 128), dtype=np.float32)
    for dj in range(KW):
        for di in range(KH):
            m = np.arange(128 - di)
            ball[m + di, dj * 128 + m] = weight[di, dj]
    return ball


def kernel(x: np.ndarray, weight: np.ndarray, bias: np.ndarray) -> np.ndarray:
    global _compiled
    x = np.ascontiguousarray(np.asarray(x, dtype=np.float32))
    weight = np.asarray(weight, dtype=np.float32)
    bias = np.asarray(bias, dtype=np.float32)

    if _compiled is None:
        _compiled = _build()
    nc = _compiled

    xpad = np.zeros((H, NCORES * COLS + KW - 1), dtype=np.float32)
    xpad[:, :W] = x
    ball = _banded(weight)
    bias_col = np.full((128, 1), bias[0], dtype=np.float32)

    in_maps = []
    for c in range(NCORES):
        in_maps.append({
            "xs": np.ascontiguousarray(xpad[:, COLS * c: COLS * c + XC]),
            "bmat": ball,
            "biast": bias_col,
        })

    res = bass_utils.run_bass_kernel_spmd(nc, in_maps,
                                          core_ids=list(range(NCORES)),
                                          trace=TRACE)
    global LAST_EXEC_NS
    LAST_EXEC_NS = res.exec_time_ns
    out = np.hstack([res.results[c]["out"] for c in range(NCORES)])
    return np.ascontiguousarray(out[:, :OW])


# revision 23
# speedup vs baseline: 1.7223x; 1.7223x over previous
"""Trainium2 Bass kernel: 4096x4096 valid 5x5 cross-correlation + scalar bias.

Strategy (8 NeuronCores, SPMD):
  - Shard the OUTPUT by columns: core c computes out[:, 512c : 512c+512]
    (core 7's last 4 columns are padding, trimmed after gather). Each core
    reads x rows 0..4095, cols [512c, 512c+516) (host-padded to width 4100).
  - On-core: the 5x5 conv is computed as banded-matrix matmuls on the
    TensorEngine. For an input row-tile X_g = x[124g : 124g+128, :] and
    kernel column dj, the banded matrix B_dj[k, m] = w[k-m, dj] gives
      (B_dj^T @ X_g[:, dj:dj+512])[m, n] = sum_di w[di, dj] x[124g+m+di, n+dj]
    so accumulating the 5 dj-matmuls in PSUM yields 124 valid output rows
    per tile. 4092 = 33 * 124 exactly; 33 tiles cover rows 0..4095 exactly.
  - PSUM accumulation is fp32. Bias is fused into the PSUM->SBUF drain
    (ScalarE Identity-activation / VectorE tensor_scalar).
  - Output DMAs are spread across the three descriptor-generation paths
    (sync HWDGE ring: 16 SDMA engines; scalar HWDGE ring: 4 engines;
    gpsimd SWDGE: descgen-limited) with a static schedule so they overlap
    the input stream, which owns the sync ring early in the kernel.
"""
import os

os.environ.setdefault("MYCRO_LOCAL_CACHE", "1")

import numpy as np

import concourse.bass as bass
import concourse.bacc as bacc
import concourse.tile as tile
import concourse.mybir as mybir
from concourse import bass_utils

H, W = 4096, 4096
KH, KW = 5, 5
OH, OW = H - KH + 1, W - KW + 1          # 4092, 4092
NCORES = 8
COLS = 512                               # output cols per core
XC = COLS + KW - 1                       # 516 input cols per core
NG = 33                                  # row tiles per core (33*124 = 4092)
RV = 124                                 # valid output rows per tile
BLK = 4                                  # tiles per PSUM block (4 of 8 banks
                                         # -> two blocks in flight, PE never
                                         # stalls on drains at block edges)

_compiled = None
TRACE = False            # test harness can flip this for neuron-profile timing
LAST_EXEC_NS = None

X_DT = "f32r"            # matmul operand dtype: "bf16" | "f32r"
STAGE_BUFS = 6
# Each group's output is written as two DMAs of 64 and 60 rows: the SDMA
# engine fan-out is the largest divisor of the partition count <= 16, so
# 64/60 rows hit 16/15 engines while the naive 124 rows would collapse to 4.
# Chunks are disjoint -> no ordering hazards. Rings rotate per group so no
# single sequencer accumulates the ~0.65us-per-push issue cost.
OUT_RING = ["scalar", "sync", "scalar", "gpsimd"]   # indexed by g % 4
OUT_SPLIT = (64, 60)


def _mm_dt():
    return mybir.dt.bfloat16 if X_DT == "bf16" else mybir.dt.float32r


def _build():
    nc = bacc.Bacc("TRN2", target_bir_lowering=False, debug=False,
                   num_devices=NCORES)
    mdt = _mm_dt()

    x_dram = nc.dram_tensor("xs", (H, XC), mdt, kind="ExternalInput")
    b_dram = nc.dram_tensor("bmat", (128, KW * 128), mdt,
                            kind="ExternalInput")
    bias_dram = nc.dram_tensor("biast", (128, 1), mybir.dt.float32,
                               kind="ExternalInput")
    out_dram = nc.dram_tensor("out", (OH, COLS), mybir.dt.float32,
                              kind="ExternalOutput")

    blocks = [list(range(s, min(s + BLK, NG))) for s in range(0, NG, BLK)]
    engs = lambda: {"scalar": nc.scalar, "sync": nc.sync, "gpsimd": nc.gpsimd}

    with tile.TileContext(nc) as tc:
        with (
            tc.tile_pool(name="const", bufs=1) as cpool,
            tc.tile_pool(name="x", bufs=NG) as xpool,
            tc.tile_pool(name="stage", bufs=STAGE_BUFS) as spool,
            tc.tile_pool(name="psum", bufs=8, space=bass.MemorySpace.PSUM) as ppool,
        ):
            bt = cpool.tile([128, KW * 128], mdt)
            biast = cpool.tile([128, 1], mybir.dt.float32)
            nc.sync.dma_start(bt[:], b_dram.ap())
            nc.sync.dma_start(biast[:], bias_dram.ap())

            # input row-tiles: tile g holds x rows [124g, 124g+128)
            xts = []
            for g in range(NG):
                xt = xpool.tile([128, XC], mdt, tag="x")
                nc.sync.dma_start(xt[:], x_dram.ap()[124 * g:124 * g + 128, :])
                xts.append(xt)

            for bi, blk in enumerate(blocks):
                stg = spool.tile([128, len(blk) * COLS], mybir.dt.float32)
                psts = {}
                for g in blk:
                    psts[g] = ppool.tile([128, COLS], mybir.dt.float32,
                                         name=f"ps{g}", tag="ps")
                # weight-stationary sweep: dj outer, tiles inner
                for dj in range(KW):
                    for g in blk:
                        nc.tensor.matmul(
                            psts[g][:],
                            bt[:, dj * 128:(dj + 1) * 128],
                            xts[g][:, dj:dj + COLS],
                            start=(dj == 0),
                            stop=(dj == KW - 1),
                        )
                # drain PSUM -> stage with fused bias, split DVE/ACT
                for i, g in enumerate(blk):
                    dst = stg[0:RV, i * COLS:(i + 1) * COLS]
                    if i % 4 < 3:
                        nc.vector.tensor_scalar_add(dst, psts[g][0:RV, :],
                                                    biast[0:RV, :])
                    else:
                        nc.scalar.activation(dst, psts[g][0:RV, :],
                                             mybir.ActivationFunctionType.Identity,
                                             bias=biast[0:RV, :])
                # output DMAs: per group, two row-chunks with 16/15-engine
                # fan-out, each a contiguous DRAM span
                for i, g in enumerate(blk):
                    ring = engs()[OUT_RING[g % len(OUT_RING)]]
                    r0 = 0
                    for rows in OUT_SPLIT:
                        ring.dma_start(
                            out_dram.ap()[124 * g + r0:124 * g + r0 + rows, :],
                            stg[r0:r0 + rows, i * COLS:(i + 1) * COLS])
                        r0 += rows

    nc.compile()
    return nc


def _banded(weight: np.ndarray) -> np.ndarray:
    ball = np.zeros((128, KW * 128), dtype=np.float32)
    for dj in range(KW):
        for di in range(KH):
            m = np.arange(128 - di)
            ball[m + di, dj * 128 + m] = weight[di, dj]
    return ball


def _to_mm_np(a: np.ndarray) -> np.ndarray:
    if X_DT == "bf16":
        import ml_dtypes
        return a.astype(ml_dtypes.bfloat16)
    return a


def kernel(x: np.ndarray, weight: np.ndarray, bias: np.ndarray) -> np.ndarray:
    global _compiled
    x = np.ascontiguousarray(np.asarray(x, dtype=np.float32))
    weight = np.asarray(weight, dtype=np.float32)
    bias = np.asarray(bias, dtype=np.float32)

    if _compiled is None:
        _compiled = _build()
    nc = _compiled

    xpad = np.zeros((H, NCORES * COLS + KW - 1), dtype=np.float32)
    xpad[:, :W] = x
    xpad = _to_mm_np(xpad)
    ball = _to_mm_np(_banded(weight))
    bias_col = np.full((128, 1), bias[0], dtype=np.float32)

    in_maps = []
    for c in range(NCORES):
        in_maps.append({
            "xs": np.ascontiguousarray(xpad[:, COLS * c: COLS * c + XC]),
            "bmat": ball,
            "biast": bias_col,
        })

    res = bass_utils.run_bass_kernel_spmd(nc, in_maps,
                                          core_ids=list(range(NCORES)),
                                          trace=TRACE)
    global LAST_EXEC_NS
    LAST_EXEC_NS = res.exec_time_ns
    out = np.hstack([res.results[c]["out"] for c in range(NCORES)])
    return np.ascontiguousarray(out[:, :OW])
